# revision 19
# baseline (speedup 1.0000x reference)
"""LIF bank (nn_LIFBank_17059610100011) Trainium2 Bass kernel.

Per-lane recurrence (T sequential steps), data-parallel over B*N lanes:
8 cores x 4096 lanes ([128 partitions, 32 free] tiles).

v5: time-major chunk layout + all-custom DVE ops + Pool offload.

- Chunks live in SBUF as [P, TC, F] (time-page major) so every per-step
  slice [:, t, :] is contiguous (strided access added ~30-100ns/op in v4).
  Host pre-transposes u to [P, T, F] per core so chunk DMAs are contiguous;
  outputs come back as [P, T, F] and are transposed on host.
- Spikes are produced complemented (ns_t = (W_t < theta_{t-1}) = 1 - s_t),
  which turns the refractory gating into plain products:
      P_{t+1} = u_{t+1} * ns_{t-1}     (Pool engine tensor_tensor mult,
                                        2 steps of slack off critical path)
      M_{t+1} = P_{t+1} * ns_t         (DVE custom PROD)
  Host recovers s = 1 - ns (exact: values are 0.0/1.0).
- Step window order on DVE (all contiguous custom ops):
      W_t   = alpha*V_{t-1} + M_t      (LIF_W)
      NS_t  = (W_t < theta_{t-1})      (LIF_LT) -> ns chunk page
      V_t   = softreset(W_t, theta)    (LIF_SOFTRESET) -> v chunk page
      T_t   = (theta*BETA + c) + GAMMA*(W_t>=theta)   (LIF_THETASPIKE)
      M_t+1 = P_{t+1} * NS_t           (LIF_PROD)

fp32 rounding order matches the jax reference exactly (mult-then-add,
two roundings; c = tb*(1-BETA) precomputed on host).
"""

import numpy as np

ALPHA = 0.95
BETA = 0.995   # THETA_DECAY
GAMMA = 0.35   # THETA_INC

B, N, T = 16, 2048, 1000
NCORES = 8
NSH = N // NCORES          # 256 neurons per core
P, F = 128, 32             # lanes per core = P*F = B*NSH = 4096
TC = 200                   # timesteps per DMA chunk

# "pool": P gating product on Pool engine; "dve": everything on DVE
P_ENGINE = "dve"

_CACHE = {}


def _register_custom_ops():
    import concourse.dve_ops as dvo
    from concourse.dve_spec import (
        Spec, Src0, Src1, C0, C1, C2, One, select, lower, _has_src1,
    )
    from concourse.dve_uop import DveOpSpec

    if "LIF_MUL_COMPL" in dvo._SUB_OPCODE_FOR_NAME:
        return {o.name: o for o in dvo.OPS if o.name.startswith("LIF_")}

    specs = {
        "LIF_MUL_COMPL": Spec(
            body=Src0 * (One - Src1),
            reference=lambda in0, in1, s0, s1, imm2: (in0 * (1.0 - in1)).astype(np.float32),
        ),
        "LIF_SOFTRESET": Spec(
            body=select(Src0 < Src1, Src0, Src0 - Src1),
            reference=lambda in0, in1, s0, s1, imm2: np.where(in0 < in1, in0, in0 - in1).astype(np.float32),
        ),
        "LIF_THETASPIKE": Spec(
            body=(Src0 * C0 + C1) + (Src1 >= Src0) * C2,
            reference=lambda in0, in1, s0, s1, imm2: (
                (in0 * np.float32(s0) + np.float32(s1))
                + (in1 >= in0).astype(np.float32) * np.float32(imm2)
            ).astype(np.float32),
        ),
        "LIF_GE": Spec(
            body=(Src0 >= Src1),
            reference=lambda in0, in1, s0, s1, imm2: (in0 >= in1).astype(np.float32),
        ),
        "LIF_W": Spec(
            body=Src0 * C0 + Src1,
            reference=lambda in0, in1, s0, s1, imm2: (
                in0 * np.float32(s0) + in1
            ).astype(np.float32),
        ),
        "LIF_LT": Spec(
            body=(Src0 < Src1),
            reference=lambda in0, in1, s0, s1, imm2: (in0 < in1).astype(np.float32),
        ),
        "LIF_PROD": Spec(
            body=Src0 * Src1,
            reference=lambda in0, in1, s0, s1, imm2: (in0 * in1).astype(np.float32),
        ),
    }
    new_ops = []
    base = len(dvo.OPS)
    for i, (name, spec) in enumerate(specs.items()):
        opcode = dvo._CUSTOM_DVE_ROW_BASE + base + i
        shas = {}
        for ver in ("v3", "v4"):
            uops = lower(spec, ver=ver)
            shas[ver] = DveOpSpec(
                name=name, opcode=opcode, uops=uops, rd1_en=_has_src1(spec)
            ).sha(ver)
        dvo._SUB_OPCODE_FOR_NAME[name] = opcode
        new_ops.append(dvo.DveOp(name, spec, subdim=False, uops_sha=shas))
    dvo.OPS.extend(new_ops)
    dvo.CUSTOM_DVE_SPECS.update({o.name: o.spec for o in new_ops})
    return {o.name: o for o in new_ops}


def _build_nc(t_total, tc, c_imm):
    import concourse.bacc as bacc
    import concourse.mybir as mybir
    import concourse.tile as tile
    from concourse.instruction_name_ordered_set import InstructionNameOrderedSet

    ops = _register_custom_ops()
    W_OP, LT, SR, TS, PROD, MC = (
        ops["LIF_W"], ops["LIF_LT"], ops["LIF_SOFTRESET"],
        ops["LIF_THETASPIKE"], ops["LIF_PROD"], ops["LIF_MUL_COMPL"],
    )

    f32 = mybir.dt.float32
    op = mybir.AluOpType

    nc = bacc.Bacc("TRN2", target_bir_lowering=False, num_devices=NCORES)
    # time-major layouts: [P, T, F]; ns and v interleave in one output tensor
    # ([..., 0:F] = ns page, [..., F:2F] = v page) -> one DMA per chunk.
    u_d = nc.dram_tensor("u", [P, t_total, F], f32, kind="ExternalInput")
    tb_d = nc.dram_tensor("tb", [P, F], f32, kind="ExternalInput")
    io_d = nc.dram_tensor("io", [P, t_total, 2 * F], f32, kind="ExternalOutput")

    nchunks = t_total // tc
    assert nchunks * tc == t_total
    vec = nc.vector
    gp = nc.gpsimd

    with tile.TileContext(nc) as tc_ctx:
        with (
            tc_ctx.tile_pool(name="state", bufs=1) as st,
            tc_ctx.tile_pool(name="ustage", bufs=2) as upool,
            tc_ctx.tile_pool(name="iostage", bufs=2) as iopool,
        ):
            one = st.tile([P, F], f32, tag="one", name="one")
            th = [st.tile([P, F], f32, tag=f"th{i}", name=f"th{i}") for i in range(4)]
            wr = [st.tile([P, F], f32, tag=f"w{i}", name=f"w{i}") for i in range(2)]
            pr = [st.tile([P, F], f32, tag=f"p{i}", name=f"p{i}") for i in range(3)]
            mr = [st.tile([P, F], f32, tag=f"m{i}", name=f"m{i}") for i in range(2)]

            vec.memset(one[:], 1.0)
            nc.sync.dma_start(th[3][:], tb_d[:, :])  # theta_{-1} = tb

            ub, iob = {}, {}

            def load_chunk(c):
                if c < nchunks and c not in ub:
                    ub[c] = upool.tile([P, tc, F], f32, tag="ub", name=f"ub{c}")
                    nc.sync.dma_start(ub[c][:], u_d[:, c * tc:(c + 1) * tc, :])

            def u_at(t):
                return ub[t // tc][:, t % tc, :]

            def ns_at(t):
                return one[:, :] if t < 0 else iob[t // tc][:, t % tc, 0:F]

            def v_at(t):
                return iob[t // tc][:, t % tc, F:2 * F]

            load_chunk(0)

            # Chain every DVE op to its predecessor with an ordering-only
            # (nosync) dependency: the Tile scheduler otherwise reorders the
            # stream and creates distance-1 RAW pairs (~+125ns each).
            _prev = [None]

            def chained(inst):
                if _prev[0] is not None:
                    deps = InstructionNameOrderedSet()
                    deps.add(_prev[0])
                    inst.ins.add_nosync_dependencies_from(deps)
                _prev[0] = inst.ins.name
                return inst

            # prologue: no spikes before t=0 -> ns_{-1} = ns_{-2} = 1:
            # P_0 = u_0, M_0 = P_0.
            chained(vec._custom_dve(PROD, out=pr[0][:], in0=u_at(0), in1=one[:, :]))
            chained(vec._custom_dve(PROD, out=mr[0][:], in0=pr[0][:], in1=one[:, :]))

            for t in range(t_total):
                c = t // tc
                if t % tc == 0:
                    iob[c] = iopool.tile([P, tc, 2 * F], f32, tag="iob", name=f"iobc{c}")
                    load_chunk(c + 1)

                thp = th[(t - 1) % 4][:, :]   # theta_{t-1}
                w = wr[t % 2][:]
                ns_page = iob[c][:, t % tc, 0:F]

                # Software-pipelined order: every op's tensor inputs are
                # produced >= 2 instructions earlier (distance-1 RAW on DVE
                # costs ~+125ns: pipeline drain + SBUF turnaround).
                #   W_t   <- V_{t-1} (d>=4), M_t (d=2)
                #   P_t+1 <- ns_{t-1} (previous step)
                #   NS_t  <- W_t (d=2)
                #   V_t   <- W_t (d=3)
                #   M_t+1 <- P_{t+1} (d=3), NS_t (d=2)
                #   T_t   <- W_t (d=5)
                # W_t = alpha*V_{t-1} + M_t  (V_{-1} = 0: use M directly)
                if t == 0:
                    chained(vec._custom_dve(W_OP, out=w, in0=mr[0][:], in1=mr[0][:], s0=0.0))
                else:
                    chained(vec._custom_dve(W_OP, out=w, in0=v_at(t - 1), in1=mr[t % 2][:], s0=ALPHA))
                # P_{t+1} = u_{t+1} * ns_{t-1}
                if t + 1 < t_total:
                    if P_ENGINE == "pool":
                        gp.tensor_tensor(
                            out=pr[(t + 1) % 3][:], in0=u_at(t + 1), in1=ns_at(t - 1),
                            op=op.mult,
                        )
                    else:
                        chained(vec._custom_dve(
                            PROD, out=pr[(t + 1) % 3][:], in0=u_at(t + 1), in1=ns_at(t - 1),
                        ))
                # NS_t = (W_t < theta_{t-1})  -> spike output (complemented)
                chained(vec._custom_dve(LT, out=ns_page, in0=w, in1=thp))
                # V_t = soft reset
                chained(vec._custom_dve(SR, out=v_at(t), in0=w, in1=thp))
                # M_{t+1} = P_{t+1} * ns_t
                if t + 1 < t_total:
                    chained(vec._custom_dve(
                        PROD, out=mr[(t + 1) % 2][:], in0=pr[(t + 1) % 3][:],
                        in1=ns_page,
                    ))
                # theta_t = (theta*BETA + c) + GAMMA*(W>=theta)
                chained(vec._custom_dve(
                    TS, out=th[t % 4][:], in0=thp, in1=w,
                    s0=BETA, s1=c_imm, imm2=GAMMA,
                ))

                if t % tc == tc - 1:
                    nc.sync.dma_start(io_d[:, c * tc:(c + 1) * tc, :], iob[c][:])

    nc.compile()
    return nc


def _get_nc(t_total, tc, c_imm):
    key = (t_total, tc, float(c_imm), P_ENGINE)
    if key not in _CACHE:
        _CACHE[key] = _build_nc(t_total, tc, c_imm)
    return _CACHE[key]


def _shard_inputs(u, theta_base, t_total):
    u = np.asarray(u, dtype=np.float32)
    tb = np.asarray(theta_base, dtype=np.float32)[0, :, 0]  # [N]
    in_maps = []
    for c in range(NCORES):
        lo, hi = c * NSH, (c + 1) * NSH
        uc = u[:, lo:hi, :t_total].reshape(P, F, t_total)
        uc = np.ascontiguousarray(uc.transpose(0, 2, 1))  # [P, T, F]
        tbc = np.tile(tb[lo:hi].reshape(NSH // F, F), (B, 1)).astype(np.float32)
        in_maps.append({"u": uc, "tb": tbc})
    return in_maps


def _unshard(res, t_total):
    s_full = np.empty((B, N, t_total), dtype=np.float32)
    v_full = np.empty((B, N, t_total), dtype=np.float32)
    for c in range(NCORES):
        lo, hi = c * NSH, (c + 1) * NSH
        io = res[c]["io"]                       # [P, T, 2F]
        ns = io[:, :, 0:F].transpose(0, 2, 1)   # [P, F, T]
        v = io[:, :, F:2 * F].transpose(0, 2, 1)
        s_full[:, lo:hi, :] = (1.0 - ns).reshape(B, NSH, t_total)
        v_full[:, lo:hi, :] = v.reshape(B, NSH, t_total)
    return s_full, v_full


def _host_fallback(u, theta_base):
    """Exact numpy step simulation; only used if theta_base is non-uniform."""
    u = np.asarray(u, np.float32)
    b, n, t = u.shape
    tb = np.asarray(theta_base, np.float32)[0, :, 0]
    v = np.zeros((b, n), np.float32)
    theta = np.broadcast_to(tb, (b, n)).astype(np.float32).copy()
    ref = np.zeros((b, n), np.float32)
    c = (tb * np.float32(1.0 - BETA)).astype(np.float32)
    ss = np.empty((b, n, t), np.float32)
    vs = np.empty((b, n, t), np.float32)
    for i in range(t):
        u_eff = np.where(ref > 0, np.float32(0.0), u[:, :, i])
        v = (np.float32(ALPHA) * v + u_eff).astype(np.float32)
        s = (v >= theta).astype(np.float32)
        v = (v - s * theta).astype(np.float32)
        ref = np.where(s > 0, np.float32(2.0), np.maximum(ref - 1.0, 0.0).astype(np.float32))
        theta = ((theta * np.float32(BETA) + c) + np.float32(GAMMA) * s).astype(np.float32)
        ss[:, :, i] = s
        vs[:, :, i] = v
    return ss, vs


def run(u, theta_base, t_total=T, tc=TC, trace=False):
    from concourse.bass_utils import run_bass_kernel_spmd

    tb = np.asarray(theta_base, dtype=np.float32)
    c_imm = float(np.float32(tb.flat[0]) * np.float32(1.0 - BETA))

    nc = _get_nc(t_total, tc, c_imm)
    in_maps = _shard_inputs(u, theta_base, t_total)
    res = run_bass_kernel_spmd(nc, in_maps, core_ids=list(range(NCORES)), trace=trace)
    s_full, v_full = _unshard(res.results, t_total)
    return (s_full, v_full), res


def kernel(u, theta_base):
    tb = np.asarray(theta_base, dtype=np.float32)
    if not np.all(tb == tb.flat[0]):
        return _host_fallback(u, theta_base)
    (s_full, v_full), _ = run(u, theta_base)
    return s_full, v_full


# revision 20
# speedup vs baseline: 1.0180x; 1.0180x over previous
"""LIF bank (nn_LIFBank_17059610100011) Trainium2 Bass kernel.

Per-lane recurrence (T sequential steps), data-parallel over B*N lanes:
8 cores x 4096 lanes ([128 partitions, 32 free] tiles).

v5: time-major chunk layout + all-custom DVE ops + Pool offload.

- Chunks live in SBUF as [P, TC, F] (time-page major) so every per-step
  slice [:, t, :] is contiguous (strided access added ~30-100ns/op in v4).
  Host pre-transposes u to [P, T, F] per core so chunk DMAs are contiguous;
  outputs come back as [P, T, F] and are transposed on host.
- Spikes are produced complemented (ns_t = (W_t < theta_{t-1}) = 1 - s_t),
  which turns the refractory gating into plain products:
      P_{t+1} = u_{t+1} * ns_{t-1}     (Pool engine tensor_tensor mult,
                                        2 steps of slack off critical path)
      M_{t+1} = P_{t+1} * ns_t         (DVE custom PROD)
  Host recovers s = 1 - ns (exact: values are 0.0/1.0).
- Step window order on DVE (all contiguous custom ops):
      W_t   = alpha*V_{t-1} + M_t      (LIF_W)
      NS_t  = (W_t < theta_{t-1})      (LIF_LT) -> ns chunk page
      V_t   = softreset(W_t, theta)    (LIF_SOFTRESET) -> v chunk page
      T_t   = (theta*BETA + c) + GAMMA*(W_t>=theta)   (LIF_THETASPIKE)
      M_t+1 = P_{t+1} * NS_t           (LIF_PROD)

fp32 rounding order matches the jax reference exactly (mult-then-add,
two roundings; c = tb*(1-BETA) precomputed on host).
"""

import numpy as np

ALPHA = 0.95
BETA = 0.995   # THETA_DECAY
GAMMA = 0.35   # THETA_INC

B, N, T = 16, 2048, 1000
NCORES = 8
NSH = N // NCORES          # 256 neurons per core
P, F = 128, 32             # lanes per core = P*F = B*NSH = 4096
TC = 125                   # timesteps per DMA chunk

# "pool": P gating product on Pool engine; "dve": everything on DVE
P_ENGINE = "dve"

_CACHE = {}


def _register_custom_ops():
    import concourse.dve_ops as dvo
    from concourse.dve_spec import (
        Spec, Src0, Src1, C0, C1, C2, One, select, lower, _has_src1,
    )
    from concourse.dve_uop import DveOpSpec

    if "LIF_MUL_COMPL" in dvo._SUB_OPCODE_FOR_NAME:
        return {o.name: o for o in dvo.OPS if o.name.startswith("LIF_")}

    specs = {
        "LIF_MUL_COMPL": Spec(
            body=Src0 * (One - Src1),
            reference=lambda in0, in1, s0, s1, imm2: (in0 * (1.0 - in1)).astype(np.float32),
        ),
        "LIF_SOFTRESET": Spec(
            body=select(Src0 < Src1, Src0, Src0 - Src1),
            reference=lambda in0, in1, s0, s1, imm2: np.where(in0 < in1, in0, in0 - in1).astype(np.float32),
        ),
        "LIF_THETASPIKE": Spec(
            body=(Src0 * C0 + C1) + (Src1 >= Src0) * C2,
            reference=lambda in0, in1, s0, s1, imm2: (
                (in0 * np.float32(s0) + np.float32(s1))
                + (in1 >= in0).astype(np.float32) * np.float32(imm2)
            ).astype(np.float32),
        ),
        "LIF_GE": Spec(
            body=(Src0 >= Src1),
            reference=lambda in0, in1, s0, s1, imm2: (in0 >= in1).astype(np.float32),
        ),
        "LIF_W": Spec(
            body=Src0 * C0 + Src1,
            reference=lambda in0, in1, s0, s1, imm2: (
                in0 * np.float32(s0) + in1
            ).astype(np.float32),
        ),
        "LIF_LT": Spec(
            body=(Src0 < Src1),
            reference=lambda in0, in1, s0, s1, imm2: (in0 < in1).astype(np.float32),
        ),
        "LIF_PROD": Spec(
            body=Src0 * Src1,
            reference=lambda in0, in1, s0, s1, imm2: (in0 * in1).astype(np.float32),
        ),
    }
    new_ops = []
    base = len(dvo.OPS)
    for i, (name, spec) in enumerate(specs.items()):
        opcode = dvo._CUSTOM_DVE_ROW_BASE + base + i
        shas = {}
        for ver in ("v3", "v4"):
            uops = lower(spec, ver=ver)
            shas[ver] = DveOpSpec(
                name=name, opcode=opcode, uops=uops, rd1_en=_has_src1(spec)
            ).sha(ver)
        dvo._SUB_OPCODE_FOR_NAME[name] = opcode
        new_ops.append(dvo.DveOp(name, spec, subdim=False, uops_sha=shas))
    dvo.OPS.extend(new_ops)
    dvo.CUSTOM_DVE_SPECS.update({o.name: o.spec for o in new_ops})
    return {o.name: o for o in new_ops}


def _build_nc(t_total, tc, c_imm):
    import concourse.bacc as bacc
    import concourse.mybir as mybir
    import concourse.tile as tile
    from concourse.instruction_name_ordered_set import InstructionNameOrderedSet

    ops = _register_custom_ops()
    W_OP, LT, SR, TS, PROD, MC = (
        ops["LIF_W"], ops["LIF_LT"], ops["LIF_SOFTRESET"],
        ops["LIF_THETASPIKE"], ops["LIF_PROD"], ops["LIF_MUL_COMPL"],
    )

    f32 = mybir.dt.float32
    op = mybir.AluOpType

    nc = bacc.Bacc("TRN2", target_bir_lowering=False, num_devices=NCORES)
    # time-major layouts: [P, T, F]; ns and v interleave in one output tensor
    # ([..., 0:F] = ns page, [..., F:2F] = v page) -> one DMA per chunk.
    u_d = nc.dram_tensor("u", [P, t_total, F], f32, kind="ExternalInput")
    tb_d = nc.dram_tensor("tb", [P, F], f32, kind="ExternalInput")
    io_d = nc.dram_tensor("io", [P, t_total, 2 * F], f32, kind="ExternalOutput")

    nchunks = t_total // tc
    assert nchunks * tc == t_total
    vec = nc.vector
    gp = nc.gpsimd

    with tile.TileContext(nc) as tc_ctx:
        with (
            tc_ctx.tile_pool(name="state", bufs=1) as st,
            tc_ctx.tile_pool(name="ustage", bufs=3) as upool,
            tc_ctx.tile_pool(name="iostage", bufs=3) as iopool,
        ):
            one = st.tile([P, F], f32, tag="one", name="one")
            th = [st.tile([P, F], f32, tag=f"th{i}", name=f"th{i}") for i in range(4)]
            wr = [st.tile([P, F], f32, tag=f"w{i}", name=f"w{i}") for i in range(2)]
            pr = [st.tile([P, F], f32, tag=f"p{i}", name=f"p{i}") for i in range(3)]
            mr = [st.tile([P, F], f32, tag=f"m{i}", name=f"m{i}") for i in range(2)]

            vec.memset(one[:], 1.0)
            nc.sync.dma_start(th[3][:], tb_d[:, :])  # theta_{-1} = tb

            ub, iob = {}, {}

            def load_chunk(c):
                if c < nchunks and c not in ub:
                    ub[c] = upool.tile([P, tc, F], f32, tag="ub", name=f"ub{c}")
                    nc.sync.dma_start(ub[c][:], u_d[:, c * tc:(c + 1) * tc, :])

            def u_at(t):
                return ub[t // tc][:, t % tc, :]

            def ns_at(t):
                return one[:, :] if t < 0 else iob[t // tc][:, t % tc, 0:F]

            def v_at(t):
                return iob[t // tc][:, t % tc, F:2 * F]

            load_chunk(0)

            # Chain every DVE op to its predecessor with an ordering-only
            # (nosync) dependency: the Tile scheduler otherwise reorders the
            # stream and creates distance-1 RAW pairs (~+125ns each).
            _prev = [None]

            def chained(inst):
                if _prev[0] is not None:
                    deps = InstructionNameOrderedSet()
                    deps.add(_prev[0])
                    inst.ins.add_nosync_dependencies_from(deps)
                _prev[0] = inst.ins.name
                return inst

            # prologue: no spikes before t=0 -> ns_{-1} = ns_{-2} = 1:
            # P_0 = u_0, M_0 = P_0.
            chained(vec._custom_dve(PROD, out=pr[0][:], in0=u_at(0), in1=one[:, :]))
            chained(vec._custom_dve(PROD, out=mr[0][:], in0=pr[0][:], in1=one[:, :]))

            for t in range(t_total):
                c = t // tc
                if t % tc == 0:
                    iob[c] = iopool.tile([P, tc, 2 * F], f32, tag="iob", name=f"iobc{c}")
                    load_chunk(c + 1)

                thp = th[(t - 1) % 4][:, :]   # theta_{t-1}
                w = wr[t % 2][:]
                ns_page = iob[c][:, t % tc, 0:F]

                # Software-pipelined order: every op's tensor inputs are
                # produced >= 2 instructions earlier (distance-1 RAW on DVE
                # costs ~+125ns: pipeline drain + SBUF turnaround).
                #   W_t   <- V_{t-1} (d>=4), M_t (d=2)
                #   P_t+1 <- ns_{t-1} (previous step)
                #   NS_t  <- W_t (d=2)
                #   V_t   <- W_t (d=3)
                #   M_t+1 <- P_{t+1} (d=3), NS_t (d=2)
                #   T_t   <- W_t (d=5)
                # W_t = alpha*V_{t-1} + M_t  (V_{-1} = 0: use M directly)
                if t == 0:
                    chained(vec._custom_dve(W_OP, out=w, in0=mr[0][:], in1=mr[0][:], s0=0.0))
                else:
                    chained(vec._custom_dve(W_OP, out=w, in0=v_at(t - 1), in1=mr[t % 2][:], s0=ALPHA))
                # P_{t+1} = u_{t+1} * ns_{t-1}
                if t + 1 < t_total:
                    if P_ENGINE == "pool":
                        gp.tensor_tensor(
                            out=pr[(t + 1) % 3][:], in0=u_at(t + 1), in1=ns_at(t - 1),
                            op=op.mult,
                        )
                    else:
                        chained(vec._custom_dve(
                            PROD, out=pr[(t + 1) % 3][:], in0=u_at(t + 1), in1=ns_at(t - 1),
                        ))
                # NS_t = (W_t < theta_{t-1})  -> spike output (complemented)
                chained(vec._custom_dve(LT, out=ns_page, in0=w, in1=thp))
                # V_t = soft reset
                chained(vec._custom_dve(SR, out=v_at(t), in0=w, in1=thp))
                # M_{t+1} = P_{t+1} * ns_t
                if t + 1 < t_total:
                    chained(vec._custom_dve(
                        PROD, out=mr[(t + 1) % 2][:], in0=pr[(t + 1) % 3][:],
                        in1=ns_page,
                    ))
                # theta_t = (theta*BETA + c) + GAMMA*(W>=theta)
                chained(vec._custom_dve(
                    TS, out=th[t % 4][:], in0=thp, in1=w,
                    s0=BETA, s1=c_imm, imm2=GAMMA,
                ))

                if t % tc == tc - 1:
                    nc.sync.dma_start(io_d[:, c * tc:(c + 1) * tc, :], iob[c][:])

    nc.compile()
    return nc


def _get_nc(t_total, tc, c_imm):
    key = (t_total, tc, float(c_imm), P_ENGINE)
    if key not in _CACHE:
        _CACHE[key] = _build_nc(t_total, tc, c_imm)
    return _CACHE[key]


def _shard_inputs(u, theta_base, t_total):
    u = np.asarray(u, dtype=np.float32)
    tb = np.asarray(theta_base, dtype=np.float32)[0, :, 0]  # [N]
    in_maps = []
    for c in range(NCORES):
        lo, hi = c * NSH, (c + 1) * NSH
        uc = u[:, lo:hi, :t_total].reshape(P, F, t_total)
        uc = np.ascontiguousarray(uc.transpose(0, 2, 1))  # [P, T, F]
        tbc = np.tile(tb[lo:hi].reshape(NSH // F, F), (B, 1)).astype(np.float32)
        in_maps.append({"u": uc, "tb": tbc})
    return in_maps


def _unshard(res, t_total):
    s_full = np.empty((B, N, t_total), dtype=np.float32)
    v_full = np.empty((B, N, t_total), dtype=np.float32)
    for c in range(NCORES):
        lo, hi = c * NSH, (c + 1) * NSH
        io = res[c]["io"]                       # [P, T, 2F]
        ns = io[:, :, 0:F].transpose(0, 2, 1)   # [P, F, T]
        v = io[:, :, F:2 * F].transpose(0, 2, 1)
        s_full[:, lo:hi, :] = (1.0 - ns).reshape(B, NSH, t_total)
        v_full[:, lo:hi, :] = v.reshape(B, NSH, t_total)
    return s_full, v_full


def _host_fallback(u, theta_base):
    """Exact numpy step simulation; only used if theta_base is non-uniform."""
    u = np.asarray(u, np.float32)
    b, n, t = u.shape
    tb = np.asarray(theta_base, np.float32)[0, :, 0]
    v = np.zeros((b, n), np.float32)
    theta = np.broadcast_to(tb, (b, n)).astype(np.float32).copy()
    ref = np.zeros((b, n), np.float32)
    c = (tb * np.float32(1.0 - BETA)).astype(np.float32)
    ss = np.empty((b, n, t), np.float32)
    vs = np.empty((b, n, t), np.float32)
    for i in range(t):
        u_eff = np.where(ref > 0, np.float32(0.0), u[:, :, i])
        v = (np.float32(ALPHA) * v + u_eff).astype(np.float32)
        s = (v >= theta).astype(np.float32)
        v = (v - s * theta).astype(np.float32)
        ref = np.where(s > 0, np.float32(2.0), np.maximum(ref - 1.0, 0.0).astype(np.float32))
        theta = ((theta * np.float32(BETA) + c) + np.float32(GAMMA) * s).astype(np.float32)
        ss[:, :, i] = s
        vs[:, :, i] = v
    return ss, vs


def run(u, theta_base, t_total=T, tc=TC, trace=False):
    from concourse.bass_utils import run_bass_kernel_spmd

    tb = np.asarray(theta_base, dtype=np.float32)
    c_imm = float(np.float32(tb.flat[0]) * np.float32(1.0 - BETA))

    nc = _get_nc(t_total, tc, c_imm)
    in_maps = _shard_inputs(u, theta_base, t_total)
    res = run_bass_kernel_spmd(nc, in_maps, core_ids=list(range(NCORES)), trace=trace)
    s_full, v_full = _unshard(res.results, t_total)
    return (s_full, v_full), res


def kernel(u, theta_base):
    tb = np.asarray(theta_base, dtype=np.float32)
    if not np.all(tb == tb.flat[0]):
        return _host_fallback(u, theta_base)
    (s_full, v_full), _ = run(u, theta_base)
    return s_full, v_full


# revision 22
# speedup vs baseline: 1.0310x; 1.0127x over previous
"""LIF bank (nn_LIFBank_17059610100011) Trainium2 Bass kernel.

Per-lane recurrence (T sequential steps), data-parallel over B*N lanes:
8 cores x 4096 lanes ([128 partitions, 32 free] tiles).

v5: time-major chunk layout + all-custom DVE ops + Pool offload.

- Chunks live in SBUF as [P, TC, F] (time-page major) so every per-step
  slice [:, t, :] is contiguous (strided access added ~30-100ns/op in v4).
  Host pre-transposes u to [P, T, F] per core so chunk DMAs are contiguous;
  outputs come back as [P, T, F] and are transposed on host.
- Spikes are produced complemented (ns_t = (W_t < theta_{t-1}) = 1 - s_t),
  which turns the refractory gating into plain products:
      P_{t+1} = u_{t+1} * ns_{t-1}     (Pool engine tensor_tensor mult,
                                        2 steps of slack off critical path)
      M_{t+1} = P_{t+1} * ns_t         (DVE custom PROD)
  Host recovers s = 1 - ns (exact: values are 0.0/1.0).
- Step window order on DVE (all contiguous custom ops):
      W_t   = alpha*V_{t-1} + M_t      (LIF_W)
      NS_t  = (W_t < theta_{t-1})      (LIF_LT) -> ns chunk page
      V_t   = softreset(W_t, theta)    (LIF_SOFTRESET) -> v chunk page
      T_t   = (theta*BETA + c) + GAMMA*(W_t>=theta)   (LIF_THETASPIKE)
      M_t+1 = P_{t+1} * NS_t           (LIF_PROD)

fp32 rounding order matches the jax reference exactly (mult-then-add,
two roundings; c = tb*(1-BETA) precomputed on host).
"""

import numpy as np

ALPHA = 0.95
BETA = 0.995   # THETA_DECAY
GAMMA = 0.35   # THETA_INC

B, N, T = 16, 2048, 1000
NCORES = 8
NSH = N // NCORES          # 256 neurons per core
P, F = 128, 32             # lanes per core = P*F = B*NSH = 4096
TC = 125                   # timesteps per DMA chunk

# "pool": P gating product on Pool engine; "dve": everything on DVE
P_ENGINE = "dve"

_CACHE = {}


def _register_custom_ops():
    import concourse.dve_ops as dvo
    from concourse.dve_spec import (
        Spec, Src0, Src1, C0, C1, C2, One, select, lower, _has_src1,
    )
    from concourse.dve_uop import DveOpSpec

    if "LIF_MUL_COMPL" in dvo._SUB_OPCODE_FOR_NAME:
        return {o.name: o for o in dvo.OPS if o.name.startswith("LIF_")}

    specs = {
        "LIF_MUL_COMPL": Spec(
            body=Src0 * (One - Src1),
            reference=lambda in0, in1, s0, s1, imm2: (in0 * (1.0 - in1)).astype(np.float32),
        ),
        "LIF_SOFTRESET": Spec(
            body=select(Src0 < Src1, Src0, Src0 - Src1),
            reference=lambda in0, in1, s0, s1, imm2: np.where(in0 < in1, in0, in0 - in1).astype(np.float32),
        ),
        "LIF_THETASPIKE": Spec(
            body=(Src0 * C0 + C1) + (Src1 >= Src0) * C2,
            reference=lambda in0, in1, s0, s1, imm2: (
                (in0 * np.float32(s0) + np.float32(s1))
                + (in1 >= in0).astype(np.float32) * np.float32(imm2)
            ).astype(np.float32),
        ),
        "LIF_GE": Spec(
            body=(Src0 >= Src1),
            reference=lambda in0, in1, s0, s1, imm2: (in0 >= in1).astype(np.float32),
        ),
        "LIF_W": Spec(
            body=Src0 * C0 + Src1,
            reference=lambda in0, in1, s0, s1, imm2: (
                in0 * np.float32(s0) + in1
            ).astype(np.float32),
        ),
        "LIF_LT": Spec(
            body=(Src0 < Src1),
            reference=lambda in0, in1, s0, s1, imm2: (in0 < in1).astype(np.float32),
        ),
        "LIF_PROD": Spec(
            body=Src0 * Src1,
            reference=lambda in0, in1, s0, s1, imm2: (in0 * in1).astype(np.float32),
        ),
    }
    new_ops = []
    base = len(dvo.OPS)
    for i, (name, spec) in enumerate(specs.items()):
        opcode = dvo._CUSTOM_DVE_ROW_BASE + base + i
        shas = {}
        for ver in ("v3", "v4"):
            uops = lower(spec, ver=ver)
            shas[ver] = DveOpSpec(
                name=name, opcode=opcode, uops=uops, rd1_en=_has_src1(spec)
            ).sha(ver)
        dvo._SUB_OPCODE_FOR_NAME[name] = opcode
        new_ops.append(dvo.DveOp(name, spec, subdim=False, uops_sha=shas))
    dvo.OPS.extend(new_ops)
    dvo.CUSTOM_DVE_SPECS.update({o.name: o.spec for o in new_ops})
    return {o.name: o for o in new_ops}


def _build_nc(t_total, tc, c_imm):
    import concourse.bacc as bacc
    import concourse.mybir as mybir
    import concourse.tile as tile
    from concourse.instruction_name_ordered_set import InstructionNameOrderedSet

    ops = _register_custom_ops()
    W_OP, LT, SR, TS, PROD, MC = (
        ops["LIF_W"], ops["LIF_LT"], ops["LIF_SOFTRESET"],
        ops["LIF_THETASPIKE"], ops["LIF_PROD"], ops["LIF_MUL_COMPL"],
    )

    f32 = mybir.dt.float32
    op = mybir.AluOpType

    nc = bacc.Bacc("TRN2", target_bir_lowering=False, num_devices=NCORES)
    # time-major layouts: [P, T, F]; ns and v interleave in one output tensor
    # ([..., 0:F] = ns page, [..., F:2F] = v page) -> one DMA per chunk.
    u_d = nc.dram_tensor("u", [P, t_total, F], f32, kind="ExternalInput")
    tb_d = nc.dram_tensor("tb", [P, F], f32, kind="ExternalInput")
    io_d = nc.dram_tensor("io", [P, t_total, 2 * F], f32, kind="ExternalOutput")

    nchunks = t_total // tc
    assert nchunks * tc == t_total
    vec = nc.vector
    gp = nc.gpsimd

    with tile.TileContext(nc) as tc_ctx:
        with (
            tc_ctx.tile_pool(name="state", bufs=1) as st,
            tc_ctx.tile_pool(name="ustage", bufs=3) as upool,
            tc_ctx.tile_pool(name="iostage", bufs=3) as iopool,
        ):
            one = st.tile([P, F], f32, tag="one", name="one")
            th = [st.tile([P, F], f32, tag=f"th{i}", name=f"th{i}") for i in range(4)]
            wr = [st.tile([P, F], f32, tag=f"w{i}", name=f"w{i}") for i in range(2)]
            pr = [st.tile([P, F], f32, tag=f"p{i}", name=f"p{i}") for i in range(3)]
            mr = [st.tile([P, F], f32, tag=f"m{i}", name=f"m{i}") for i in range(2)]

            vec.memset(one[:], 1.0)
            nc.sync.dma_start(th[3][:], tb_d[:, :])  # theta_{-1} = tb

            ub, iob = {}, {}
            SUB = 25                      # DMA sub-chunk granularity (steps)
            assert tc % SUB == 0

            def load_chunk(c):
                # u loads in SUB-step pieces so compute can start ~1.2us after
                # launch instead of waiting for a full chunk.
                if c < nchunks and c not in ub:
                    ub[c] = upool.tile([P, tc, F], f32, tag="ub", name=f"ub{c}")
                    for k in range(tc // SUB):
                        nc.sync.dma_start(
                            ub[c][:, k * SUB:(k + 1) * SUB, :],
                            u_d[:, c * tc + k * SUB:c * tc + (k + 1) * SUB, :],
                        )

            def u_at(t):
                return ub[t // tc][:, t % tc, :]

            def ns_at(t):
                return one[:, :] if t < 0 else iob[t // tc][:, t % tc, 0:F]

            def v_at(t):
                return iob[t // tc][:, t % tc, F:2 * F]

            load_chunk(0)

            # Chain every DVE op to its predecessor with an ordering-only
            # (nosync) dependency: the Tile scheduler otherwise reorders the
            # stream and creates distance-1 RAW pairs (~+125ns each).
            _prev = [None]

            def chained(inst):
                if _prev[0] is not None:
                    deps = InstructionNameOrderedSet()
                    deps.add(_prev[0])
                    inst.ins.add_nosync_dependencies_from(deps)
                _prev[0] = inst.ins.name
                return inst

            # prologue: no spikes before t=0 -> ns_{-1} = ns_{-2} = 1:
            # P_0 = u_0, M_0 = P_0.
            chained(vec._custom_dve(PROD, out=pr[0][:], in0=u_at(0), in1=one[:, :]))
            chained(vec._custom_dve(PROD, out=mr[0][:], in0=pr[0][:], in1=one[:, :]))

            for t in range(t_total):
                c = t // tc
                if t % tc == 0:
                    iob[c] = iopool.tile([P, tc, 2 * F], f32, tag="iob", name=f"iobc{c}")
                    load_chunk(c + 1)

                thp = th[(t - 1) % 4][:, :]   # theta_{t-1}
                w = wr[t % 2][:]
                ns_page = iob[c][:, t % tc, 0:F]

                # Software-pipelined order: every op's tensor inputs are
                # produced >= 2 instructions earlier (distance-1 RAW on DVE
                # costs ~+125ns: pipeline drain + SBUF turnaround).
                #   W_t   <- V_{t-1} (d>=4), M_t (d=2)
                #   P_t+1 <- ns_{t-1} (previous step)
                #   NS_t  <- W_t (d=2)
                #   V_t   <- W_t (d=3)
                #   M_t+1 <- P_{t+1} (d=3), NS_t (d=2)
                #   T_t   <- W_t (d=5)
                # W_t = alpha*V_{t-1} + M_t  (V_{-1} = 0: use M directly)
                if t == 0:
                    chained(vec._custom_dve(W_OP, out=w, in0=mr[0][:], in1=mr[0][:], s0=0.0))
                else:
                    chained(vec._custom_dve(W_OP, out=w, in0=v_at(t - 1), in1=mr[t % 2][:], s0=ALPHA))
                # P_{t+1} = u_{t+1} * ns_{t-1}
                if t + 1 < t_total:
                    if P_ENGINE == "pool":
                        gp.tensor_tensor(
                            out=pr[(t + 1) % 3][:], in0=u_at(t + 1), in1=ns_at(t - 1),
                            op=op.mult,
                        )
                    else:
                        chained(vec._custom_dve(
                            PROD, out=pr[(t + 1) % 3][:], in0=u_at(t + 1), in1=ns_at(t - 1),
                        ))
                # NS_t = (W_t < theta_{t-1})  -> spike output (complemented)
                chained(vec._custom_dve(LT, out=ns_page, in0=w, in1=thp))
                # V_t = soft reset
                chained(vec._custom_dve(SR, out=v_at(t), in0=w, in1=thp))
                # M_{t+1} = P_{t+1} * ns_t
                if t + 1 < t_total:
                    chained(vec._custom_dve(
                        PROD, out=mr[(t + 1) % 2][:], in0=pr[(t + 1) % 3][:],
                        in1=ns_page,
                    ))
                # theta_t = (theta*BETA + c) + GAMMA*(W>=theta)
                chained(vec._custom_dve(
                    TS, out=th[t % 4][:], in0=thp, in1=w,
                    s0=BETA, s1=c_imm, imm2=GAMMA,
                ))

                # store finished SUB-step spans as they complete so the final
                # chunk's output DMA overlaps compute (saves ~9us of drain).
                if t % SUB == SUB - 1:
                    lo = (t // SUB) * SUB
                    nc.sync.dma_start(
                        io_d[:, lo:lo + SUB, :],
                        iob[c][:, (lo - c * tc):(lo - c * tc) + SUB, :],
                    )

    nc.compile()
    return nc


def _get_nc(t_total, tc, c_imm):
    key = (t_total, tc, float(c_imm), P_ENGINE)
    if key not in _CACHE:
        _CACHE[key] = _build_nc(t_total, tc, c_imm)
    return _CACHE[key]


def _shard_inputs(u, theta_base, t_total):
    u = np.asarray(u, dtype=np.float32)
    tb = np.asarray(theta_base, dtype=np.float32)[0, :, 0]  # [N]
    in_maps = []
    for c in range(NCORES):
        lo, hi = c * NSH, (c + 1) * NSH
        uc = u[:, lo:hi, :t_total].reshape(P, F, t_total)
        uc = np.ascontiguousarray(uc.transpose(0, 2, 1))  # [P, T, F]
        tbc = np.tile(tb[lo:hi].reshape(NSH // F, F), (B, 1)).astype(np.float32)
        in_maps.append({"u": uc, "tb": tbc})
    return in_maps


def _unshard(res, t_total):
    s_full = np.empty((B, N, t_total), dtype=np.float32)
    v_full = np.empty((B, N, t_total), dtype=np.float32)
    for c in range(NCORES):
        lo, hi = c * NSH, (c + 1) * NSH
        io = res[c]["io"]                       # [P, T, 2F]
        ns = io[:, :, 0:F].transpose(0, 2, 1)   # [P, F, T]
        v = io[:, :, F:2 * F].transpose(0, 2, 1)
        s_full[:, lo:hi, :] = (1.0 - ns).reshape(B, NSH, t_total)
        v_full[:, lo:hi, :] = v.reshape(B, NSH, t_total)
    return s_full, v_full


def _host_fallback(u, theta_base):
    """Exact numpy step simulation; only used if theta_base is non-uniform."""
    u = np.asarray(u, np.float32)
    b, n, t = u.shape
    tb = np.asarray(theta_base, np.float32)[0, :, 0]
    v = np.zeros((b, n), np.float32)
    theta = np.broadcast_to(tb, (b, n)).astype(np.float32).copy()
    ref = np.zeros((b, n), np.float32)
    c = (tb * np.float32(1.0 - BETA)).astype(np.float32)
    ss = np.empty((b, n, t), np.float32)
    vs = np.empty((b, n, t), np.float32)
    for i in range(t):
        u_eff = np.where(ref > 0, np.float32(0.0), u[:, :, i])
        v = (np.float32(ALPHA) * v + u_eff).astype(np.float32)
        s = (v >= theta).astype(np.float32)
        v = (v - s * theta).astype(np.float32)
        ref = np.where(s > 0, np.float32(2.0), np.maximum(ref - 1.0, 0.0).astype(np.float32))
        theta = ((theta * np.float32(BETA) + c) + np.float32(GAMMA) * s).astype(np.float32)
        ss[:, :, i] = s
        vs[:, :, i] = v
    return ss, vs


def run(u, theta_base, t_total=T, tc=TC, trace=False):
    from concourse.bass_utils import run_bass_kernel_spmd

    tb = np.asarray(theta_base, dtype=np.float32)
    c_imm = float(np.float32(tb.flat[0]) * np.float32(1.0 - BETA))

    nc = _get_nc(t_total, tc, c_imm)
    in_maps = _shard_inputs(u, theta_base, t_total)
    res = run_bass_kernel_spmd(nc, in_maps, core_ids=list(range(NCORES)), trace=trace)
    s_full, v_full = _unshard(res.results, t_total)
    return (s_full, v_full), res


def kernel(u, theta_base):
    tb = np.asarray(theta_base, dtype=np.float32)
    if not np.all(tb == tb.flat[0]):
        return _host_fallback(u, theta_base)
    (s_full, v_full), _ = run(u, theta_base)
    return s_full, v_full


# revision 24
# speedup vs baseline: 1.0379x; 1.0067x over previous
"""LIF bank (nn_LIFBank_17059610100011) Trainium2 Bass kernel.

Per-lane recurrence (T sequential steps), data-parallel over B*N lanes:
8 cores x 4096 lanes ([128 partitions, 32 free] tiles).

v5: time-major chunk layout + all-custom DVE ops + Pool offload.

- Chunks live in SBUF as [P, TC, F] (time-page major) so every per-step
  slice [:, t, :] is contiguous (strided access added ~30-100ns/op in v4).
  Host pre-transposes u to [P, T, F] per core so chunk DMAs are contiguous;
  outputs come back as [P, T, F] and are transposed on host.
- Spikes are produced complemented (ns_t = (W_t < theta_{t-1}) = 1 - s_t),
  which turns the refractory gating into plain products:
      P_{t+1} = u_{t+1} * ns_{t-1}     (Pool engine tensor_tensor mult,
                                        2 steps of slack off critical path)
      M_{t+1} = P_{t+1} * ns_t         (DVE custom PROD)
  Host recovers s = 1 - ns (exact: values are 0.0/1.0).
- Step window order on DVE (all contiguous custom ops):
      W_t   = alpha*V_{t-1} + M_t      (LIF_W)
      NS_t  = (W_t < theta_{t-1})      (LIF_LT) -> ns chunk page
      V_t   = softreset(W_t, theta)    (LIF_SOFTRESET) -> v chunk page
      T_t   = (theta*BETA + c) + GAMMA*(W_t>=theta)   (LIF_THETASPIKE)
      M_t+1 = P_{t+1} * NS_t           (LIF_PROD)

fp32 rounding order matches the jax reference exactly (mult-then-add,
two roundings; c = tb*(1-BETA) precomputed on host).
"""

import numpy as np

ALPHA = 0.95
BETA = 0.995   # THETA_DECAY
GAMMA = 0.35   # THETA_INC

B, N, T = 16, 2048, 1000
NCORES = 8
NSH = N // NCORES          # 256 neurons per core
P, F = 128, 32             # lanes per core = P*F = B*NSH = 4096
TC = 125                   # timesteps per DMA chunk

# "pool": P gating product on Pool engine; "dve": everything on DVE
P_ENGINE = "dve"

_CACHE = {}


def _register_custom_ops():
    import concourse.dve_ops as dvo
    from concourse.dve_spec import (
        Spec, Src0, Src1, C0, C1, C2, One, select, lower, _has_src1,
    )
    from concourse.dve_uop import DveOpSpec

    if "LIF_MUL_COMPL" in dvo._SUB_OPCODE_FOR_NAME:
        return {o.name: o for o in dvo.OPS if o.name.startswith("LIF_")}

    specs = {
        "LIF_MUL_COMPL": Spec(
            body=Src0 * (One - Src1),
            reference=lambda in0, in1, s0, s1, imm2: (in0 * (1.0 - in1)).astype(np.float32),
        ),
        "LIF_SOFTRESET": Spec(
            body=select(Src0 < Src1, Src0, Src0 - Src1),
            reference=lambda in0, in1, s0, s1, imm2: np.where(in0 < in1, in0, in0 - in1).astype(np.float32),
        ),
        "LIF_THETASPIKE": Spec(
            body=(Src0 * C0 + C1) + (Src1 >= Src0) * C2,
            reference=lambda in0, in1, s0, s1, imm2: (
                (in0 * np.float32(s0) + np.float32(s1))
                + (in1 >= in0).astype(np.float32) * np.float32(imm2)
            ).astype(np.float32),
        ),
        "LIF_GE": Spec(
            body=(Src0 >= Src1),
            reference=lambda in0, in1, s0, s1, imm2: (in0 >= in1).astype(np.float32),
        ),
        "LIF_W": Spec(
            body=Src0 * C0 + Src1,
            reference=lambda in0, in1, s0, s1, imm2: (
                in0 * np.float32(s0) + in1
            ).astype(np.float32),
        ),
        "LIF_LT": Spec(
            body=(Src0 < Src1),
            reference=lambda in0, in1, s0, s1, imm2: (in0 < in1).astype(np.float32),
        ),
        "LIF_PROD": Spec(
            body=Src0 * Src1,
            reference=lambda in0, in1, s0, s1, imm2: (in0 * in1).astype(np.float32),
        ),
    }
    new_ops = []
    base = len(dvo.OPS)
    for i, (name, spec) in enumerate(specs.items()):
        opcode = dvo._CUSTOM_DVE_ROW_BASE + base + i
        shas = {}
        for ver in ("v3", "v4"):
            uops = lower(spec, ver=ver)
            shas[ver] = DveOpSpec(
                name=name, opcode=opcode, uops=uops, rd1_en=_has_src1(spec)
            ).sha(ver)
        dvo._SUB_OPCODE_FOR_NAME[name] = opcode
        new_ops.append(dvo.DveOp(name, spec, subdim=False, uops_sha=shas))
    dvo.OPS.extend(new_ops)
    dvo.CUSTOM_DVE_SPECS.update({o.name: o.spec for o in new_ops})
    return {o.name: o for o in new_ops}


def _build_nc(t_total, tc, c_imm):
    import concourse.bacc as bacc
    import concourse.mybir as mybir
    import concourse.tile as tile
    from concourse.instruction_name_ordered_set import InstructionNameOrderedSet

    ops = _register_custom_ops()
    W_OP, LT, SR, TS, PROD, MC = (
        ops["LIF_W"], ops["LIF_LT"], ops["LIF_SOFTRESET"],
        ops["LIF_THETASPIKE"], ops["LIF_PROD"], ops["LIF_MUL_COMPL"],
    )

    f32 = mybir.dt.float32
    op = mybir.AluOpType

    nc = bacc.Bacc("TRN2", target_bir_lowering=False, num_devices=NCORES)
    # time-major layouts: [P, T, F]; ns and v interleave in one output tensor
    # ([..., 0:F] = ns page, [..., F:2F] = v page) -> one DMA per chunk.
    u_d = nc.dram_tensor("u", [P, t_total, F], f32, kind="ExternalInput")
    tb_d = nc.dram_tensor("tb", [P, F], f32, kind="ExternalInput")
    io_d = nc.dram_tensor("io", [P, t_total, 2 * F], f32, kind="ExternalOutput")

    nchunks = t_total // tc
    assert nchunks * tc == t_total
    vec = nc.vector
    gp = nc.gpsimd

    with tile.TileContext(nc) as tc_ctx:
        with (
            tc_ctx.tile_pool(name="state", bufs=1) as st,
            tc_ctx.tile_pool(name="ustage", bufs=3) as upool,
            tc_ctx.tile_pool(name="iostage", bufs=3) as iopool,
        ):
            one = st.tile([P, F], f32, tag="one", name="one")
            th = [st.tile([P, F], f32, tag=f"th{i}", name=f"th{i}") for i in range(4)]
            wr = [st.tile([P, F], f32, tag=f"w{i}", name=f"w{i}") for i in range(2)]
            pr = [st.tile([P, F], f32, tag=f"p{i}", name=f"p{i}") for i in range(3)]
            mr = [st.tile([P, F], f32, tag=f"m{i}", name=f"m{i}") for i in range(2)]

            vec.memset(one[:], 1.0)
            nc.sync.dma_start(th[3][:], tb_d[:, :])  # theta_{-1} = tb

            ub, iob = {}, {}
            SUB = 25                      # DMA sub-chunk granularity (steps)
            assert tc % SUB == 0

            def load_chunk(c):
                # u loads in SUB-step pieces so compute can start ~1.2us after
                # launch instead of waiting for a full chunk; the very first
                # piece is split finer still to cut the cold-start wait.
                if c < nchunks and c not in ub:
                    ub[c] = upool.tile([P, tc, F], f32, tag="ub", name=f"ub{c}")
                    edges = list(range(0, tc + 1, SUB))
                    if c == 0:
                        edges = [0, 5, 10, SUB] + edges[2:]
                    for lo, hi in zip(edges[:-1], edges[1:]):
                        nc.sync.dma_start(
                            ub[c][:, lo:hi, :],
                            u_d[:, c * tc + lo:c * tc + hi, :],
                        )

            def u_at(t):
                return ub[t // tc][:, t % tc, :]

            def ns_at(t):
                return one[:, :] if t < 0 else iob[t // tc][:, t % tc, 0:F]

            def v_at(t):
                return iob[t // tc][:, t % tc, F:2 * F]

            load_chunk(0)

            # Chain every DVE op to its predecessor with an ordering-only
            # (nosync) dependency: the Tile scheduler otherwise reorders the
            # stream and creates distance-1 RAW pairs (~+125ns each).
            _prev = [None]

            def chained(inst):
                if _prev[0] is not None:
                    deps = InstructionNameOrderedSet()
                    deps.add(_prev[0])
                    inst.ins.add_nosync_dependencies_from(deps)
                _prev[0] = inst.ins.name
                return inst

            # prologue: no spikes before t=0 -> ns_{-1} = ns_{-2} = 1:
            # P_0 = u_0, M_0 = P_0.
            chained(vec._custom_dve(PROD, out=pr[0][:], in0=u_at(0), in1=one[:, :]))
            chained(vec._custom_dve(PROD, out=mr[0][:], in0=pr[0][:], in1=one[:, :]))

            for t in range(t_total):
                c = t // tc
                if t % tc == 0:
                    iob[c] = iopool.tile([P, tc, 2 * F], f32, tag="iob", name=f"iobc{c}")
                    load_chunk(c + 1)

                thp = th[(t - 1) % 4][:, :]   # theta_{t-1}
                w = wr[t % 2][:]
                ns_page = iob[c][:, t % tc, 0:F]

                # Software-pipelined order: every op's tensor inputs are
                # produced >= 2 instructions earlier (distance-1 RAW on DVE
                # costs ~+125ns: pipeline drain + SBUF turnaround).
                #   W_t   <- V_{t-1} (d>=4), M_t (d=2)
                #   P_t+1 <- ns_{t-1} (previous step)
                #   NS_t  <- W_t (d=2)
                #   V_t   <- W_t (d=3)
                #   M_t+1 <- P_{t+1} (d=3), NS_t (d=2)
                #   T_t   <- W_t (d=5)
                # W_t = alpha*V_{t-1} + M_t  (V_{-1} = 0: use M directly)
                if t == 0:
                    chained(vec._custom_dve(W_OP, out=w, in0=mr[0][:], in1=mr[0][:], s0=0.0))
                else:
                    chained(vec._custom_dve(W_OP, out=w, in0=v_at(t - 1), in1=mr[t % 2][:], s0=ALPHA))
                # P_{t+1} = u_{t+1} * ns_{t-1}
                if t + 1 < t_total:
                    if P_ENGINE == "pool":
                        gp.tensor_tensor(
                            out=pr[(t + 1) % 3][:], in0=u_at(t + 1), in1=ns_at(t - 1),
                            op=op.mult,
                        )
                    else:
                        chained(vec._custom_dve(
                            PROD, out=pr[(t + 1) % 3][:], in0=u_at(t + 1), in1=ns_at(t - 1),
                        ))
                # NS_t = (W_t < theta_{t-1})  -> spike output (complemented)
                chained(vec._custom_dve(LT, out=ns_page, in0=w, in1=thp))
                # V_t = soft reset
                chained(vec._custom_dve(SR, out=v_at(t), in0=w, in1=thp))
                # M_{t+1} = P_{t+1} * ns_t
                if t + 1 < t_total:
                    chained(vec._custom_dve(
                        PROD, out=mr[(t + 1) % 2][:], in0=pr[(t + 1) % 3][:],
                        in1=ns_page,
                    ))
                # theta_t = (theta*BETA + c) + GAMMA*(W>=theta)
                chained(vec._custom_dve(
                    TS, out=th[t % 4][:], in0=thp, in1=w,
                    s0=BETA, s1=c_imm, imm2=GAMMA,
                ))

                # store finished SUB-step spans as they complete so the final
                # chunk's output DMA overlaps compute (saves ~9us of drain);
                # the tail of the last chunk drains in 5-step pieces.
                near_end = t >= t_total - SUB
                if (not near_end and t % SUB == SUB - 1) or (near_end and t % 5 == 4):
                    sz = 5 if near_end else SUB
                    lo = (t // sz) * sz
                    nc.sync.dma_start(
                        io_d[:, lo:lo + sz, :],
                        iob[c][:, (lo - c * tc):(lo - c * tc) + sz, :],
                    )

    nc.compile()
    return nc


def _get_nc(t_total, tc, c_imm):
    key = (t_total, tc, float(c_imm), P_ENGINE)
    if key not in _CACHE:
        _CACHE[key] = _build_nc(t_total, tc, c_imm)
    return _CACHE[key]


def _shard_inputs(u, theta_base, t_total):
    u = np.asarray(u, dtype=np.float32)
    tb = np.asarray(theta_base, dtype=np.float32)[0, :, 0]  # [N]
    in_maps = []
    for c in range(NCORES):
        lo, hi = c * NSH, (c + 1) * NSH
        uc = u[:, lo:hi, :t_total].reshape(P, F, t_total)
        uc = np.ascontiguousarray(uc.transpose(0, 2, 1))  # [P, T, F]
        tbc = np.tile(tb[lo:hi].reshape(NSH // F, F), (B, 1)).astype(np.float32)
        in_maps.append({"u": uc, "tb": tbc})
    return in_maps


def _unshard(res, t_total):
    s_full = np.empty((B, N, t_total), dtype=np.float32)
    v_full = np.empty((B, N, t_total), dtype=np.float32)
    for c in range(NCORES):
        lo, hi = c * NSH, (c + 1) * NSH
        io = res[c]["io"]                       # [P, T, 2F]
        ns = io[:, :, 0:F].transpose(0, 2, 1)   # [P, F, T]
        v = io[:, :, F:2 * F].transpose(0, 2, 1)
        s_full[:, lo:hi, :] = (1.0 - ns).reshape(B, NSH, t_total)
        v_full[:, lo:hi, :] = v.reshape(B, NSH, t_total)
    return s_full, v_full


def _host_fallback(u, theta_base):
    """Exact numpy step simulation; only used if theta_base is non-uniform."""
    u = np.asarray(u, np.float32)
    b, n, t = u.shape
    tb = np.asarray(theta_base, np.float32)[0, :, 0]
    v = np.zeros((b, n), np.float32)
    theta = np.broadcast_to(tb, (b, n)).astype(np.float32).copy()
    ref = np.zeros((b, n), np.float32)
    c = (tb * np.float32(1.0 - BETA)).astype(np.float32)
    ss = np.empty((b, n, t), np.float32)
    vs = np.empty((b, n, t), np.float32)
    for i in range(t):
        u_eff = np.where(ref > 0, np.float32(0.0), u[:, :, i])
        v = (np.float32(ALPHA) * v + u_eff).astype(np.float32)
        s = (v >= theta).astype(np.float32)
        v = (v - s * theta).astype(np.float32)
        ref = np.where(s > 0, np.float32(2.0), np.maximum(ref - 1.0, 0.0).astype(np.float32))
        theta = ((theta * np.float32(BETA) + c) + np.float32(GAMMA) * s).astype(np.float32)
        ss[:, :, i] = s
        vs[:, :, i] = v
    return ss, vs


def run(u, theta_base, t_total=T, tc=TC, trace=False):
    from concourse.bass_utils import run_bass_kernel_spmd

    tb = np.asarray(theta_base, dtype=np.float32)
    c_imm = float(np.float32(tb.flat[0]) * np.float32(1.0 - BETA))

    nc = _get_nc(t_total, tc, c_imm)
    in_maps = _shard_inputs(u, theta_base, t_total)
    res = run_bass_kernel_spmd(nc, in_maps, core_ids=list(range(NCORES)), trace=trace)
    s_full, v_full = _unshard(res.results, t_total)
    return (s_full, v_full), res


def kernel(u, theta_base):
    tb = np.asarray(theta_base, dtype=np.float32)
    if not np.all(tb == tb.flat[0]):
        return _host_fallback(u, theta_base)
    (s_full, v_full), _ = run(u, theta_base)
    return s_full, v_full
